# revision 2
# baseline (speedup 1.0000x reference)
"""Bilateral blur (7x7, L1 color distance) on 8 Trainium2 NeuronCores.

Input  (4, 3, 512, 512) fp32  ->  Output (4, 3, 512, 512) fp32.

Sharding: pure data parallel. Core i handles batch i//2, row-half i%2
(256 output rows x 512 cols). The host reflect-pads each image and
re-lays the shard into a "transposed-blocked" layout:

  partition p (0..127) owns output columns [4p, 4p+4); its free dim
  holds, per channel, a (10 x 262) patch [xl, y] = padded cols
  4p..4p+9 x 262 padded rows, flattened as xl*262 + y. The three
  channels are stacked along the free dim.

With that layout every one of the 49 tap shifts (dy, dx) is a pure
free-dim AP offset (dx*262 + dy) - TRN2 compute engines cannot read
across partitions (start partition must be 0/32/64/96). All shard
pixels are processed by ops of shape [128, 3*1024] / [128, 1024] in a
single pass.

Per tap k=(dy,dx) with space weight s_k (Gaussian, folded into the
accumulation ops' immediate-scalar slot):
  dlt   = taps - center          (1 fat TT over 3 channels)
  adl   = |dlt|                  (ACT Abs, parallel engine)
  dsum  = adl0 + adl1 + adl2     (2 TT)
  qb    = -50 * dsum^2           (1 STT: (d mult -50) mult d)
  wgtb  = exp(qb)                (ACT Exp)
  den  += s_k * wgtb             (1 STT: (w mult s_k) add den)
  num  += s_k * wgtb * taps      (TT mult + STT per channel group)
Final: out_c = num_c * reciprocal(den).
"""
import numpy as np

import concourse.bass as bass
import concourse.bacc as bacc
import concourse.mybir as mybir
from concourse.tile import TileContext
from concourse import bass_utils

C = 3
B, H, W = 4, 512, 512
KX = KY = 7
PAD = 3
SIGMA_COLOR = 0.1
N_CORES = 8

ROWS = 256                  # output rows per core
WG = 4                      # output cols per partition
NPART = 128
XE, YE = WG + 2 * PAD, ROWS + 2 * PAD     # 10, 262
FREE_IN = XE * YE           # 2620
FREE_OUT = WG * ROWS        # 1024


def _space_kernel():
    def g1(k, sigma):
        x = np.arange(k, dtype=np.float64) - (k - 1) / 2.0
        g = np.exp(-0.5 * (x / sigma) ** 2)
        return g / g.sum()
    gy, gx = g1(KY, 1.5), g1(KX, 1.5)
    return (gy[:, None] * gx[None, :]).reshape(-1)   # index dy*KX+dx


def _tap_ap(t, dx, dy, nch=C, ch0=0):
    a = t[:]
    return bass.AP(a.tensor, a.offset + ch0 * FREE_IN + dx * YE + dy,
                   [[C * FREE_IN, NPART], [FREE_IN, nch], [YE, WG], [1, ROWS]])


def _stk_ap(t, nch=C, ch0=0, step0=False):
    a = t[:]
    tot = a.shape[1]
    return bass.AP(a.tensor, a.offset + ch0 * FREE_OUT,
                   [[tot, NPART], [0 if step0 else FREE_OUT, nch], [ROWS, WG], [1, ROWS]])


def _build():
    nc = bacc.Bacc()
    dt = mybir.dt.float32
    x = nc.dram_tensor("x", [NPART, C * FREE_IN], dt, kind="ExternalInput")
    o = nc.dram_tensor("o", [NPART, C * FREE_OUT], dt, kind="ExternalOutput")
    s = _space_kernel()
    AOT = mybir.AluOpType
    F = FREE_OUT

    with TileContext(nc) as tc:
        with tc.tile_pool(name="persist", bufs=1) as pool, \
             tc.tile_pool(name="tmp", bufs=3) as tp:
            T = pool.tile([NPART, C * FREE_IN], dt, name="T")
            nc.sync.dma_start(T[:], x[:])

            num = pool.tile([NPART, C * F], dt, name="num")
            den = pool.tile([NPART, F], dt, name="den")
            qb = pool.tile([NPART, F], dt, name="qb")
            wgtb = pool.tile([NPART, F], dt, name="wgtb")

            ctr = _tap_ap(T, PAD, PAD)
            for dy in range(KY):
                for dx in range(KX):
                    k = dy * KX + dx
                    sk = float(s[k])
                    dlt = tp.tile([NPART, C * F], dt, name="dlt", tag="dlt")
                    nc.vector.tensor_tensor(out=_stk_ap(dlt), in0=_tap_ap(T, dx, dy),
                                            in1=ctr, op=AOT.subtract)
                    adl = tp.tile([NPART, C * F], dt, name="adl", tag="adl")
                    nc.scalar.activation(adl[:], dlt[:],
                                         mybir.ActivationFunctionType.Abs,
                                         bias=0.0, scale=1.0)
                    dsum = tp.tile([NPART, F], dt, name="dsum", tag="dsum")
                    nc.vector.tensor_tensor(out=dsum[:], in0=adl[:, 0:F],
                                            in1=adl[:, F:2 * F], op=AOT.add)
                    nc.vector.tensor_tensor(out=dsum[:], in0=dsum[:],
                                            in1=adl[:, 2 * F:], op=AOT.add)
                    nc.vector.scalar_tensor_tensor(
                        out=qb[:], in0=dsum[:], scalar=-0.5 / SIGMA_COLOR ** 2,
                        in1=dsum[:], op0=AOT.mult, op1=AOT.mult)
                    nc.scalar.activation(wgtb[:], qb[:],
                                         mybir.ActivationFunctionType.Exp,
                                         bias=0.0, scale=1.0)
                    if k == 0:
                        nc.vector.tensor_scalar(out=den[:], in0=wgtb[:], scalar1=sk,
                                                scalar2=None, op0=AOT.mult)
                    else:
                        nc.vector.scalar_tensor_tensor(
                            out=den[:], in0=wgtb[:], scalar=sk, in1=den[:],
                            op0=AOT.mult, op1=AOT.add)
                    # channels 0..1 fat; channel 2's product overwrites wgtb
                    # (flat DVE write keeps wgtb's history DVE-only)
                    m01 = tp.tile([NPART, 2 * F], dt, name="m01", tag="m01")
                    nc.vector.tensor_tensor(out=_stk_ap(m01, nch=2),
                                            in0=_stk_ap(wgtb, nch=2, step0=True),
                                            in1=_tap_ap(T, dx, dy, nch=2), op=AOT.mult)
                    nc.vector.tensor_tensor(out=wgtb[:], in0=wgtb[:],
                                            in1=_tap_ap(T, dx, dy, nch=1, ch0=2),
                                            op=AOT.mult)
                    if k == 0:
                        nc.vector.tensor_scalar(out=num[:, 0:2 * F], in0=m01[:],
                                                scalar1=sk, scalar2=None, op0=AOT.mult)
                        nc.vector.tensor_scalar(out=num[:, 2 * F:], in0=wgtb[:],
                                                scalar1=sk, scalar2=None, op0=AOT.mult)
                    else:
                        nc.vector.scalar_tensor_tensor(
                            out=num[:, 0:2 * F], in0=m01[:], scalar=sk,
                            in1=num[:, 0:2 * F], op0=AOT.mult, op1=AOT.add)
                        nc.vector.scalar_tensor_tensor(
                            out=num[:, 2 * F:], in0=wgtb[:], scalar=sk,
                            in1=num[:, 2 * F:], op0=AOT.mult, op1=AOT.add)

            recip = pool.tile([NPART, F], dt, name="recip")
            nc.vector.reciprocal(recip[:], den[:])
            ot = pool.tile([NPART, C * F], dt, name="ot")
            nc.vector.tensor_tensor(out=_stk_ap(ot), in0=_stk_ap(num),
                                    in1=_stk_ap(recip, step0=True), op=AOT.mult)
            nc.sync.dma_start(o[:], ot[:])
    return nc


_COLIDX = np.arange(NPART)[:, None] * WG + np.arange(XE)[None, :]   # (128, 10)


def _shard_layout(shard):
    """(3, 262, 518) padded shard -> [128, 3*2620] device layout."""
    buf = np.empty((NPART, C, XE, YE), np.float32)
    for c in range(C):
        buf[:, c] = shard[c].T[_COLIDX]        # (128, 10, 262)
    return buf.reshape(NPART, C * FREE_IN)


_NC_CACHE = {}


def _get_nc():
    if "nc" not in _NC_CACHE:
        nc = _build()
        nc.finalize()
        _NC_CACHE["nc"] = nc
    return _NC_CACHE["nc"]


def kernel(input: np.ndarray) -> np.ndarray:
    x = np.asarray(input, dtype=np.float32)
    assert x.shape == (B, C, H, W)
    xp = np.pad(x, ((0, 0), (0, 0), (PAD, PAD), (PAD, PAD)), mode="reflect")

    in_maps = []
    for core in range(N_CORES):
        b, half = core // 2, core % 2
        r0 = half * ROWS
        shard = xp[b, :, r0:r0 + ROWS + 2 * PAD, :]     # (3, 262, 518)
        in_maps.append({"x": _shard_layout(shard)})

    nc = _get_nc()
    res = bass_utils.run_bass_kernel_spmd(nc, in_maps, list(range(N_CORES)))

    out = np.empty((B, C, H, W), np.float32)
    for core in range(N_CORES):
        b, half = core // 2, core % 2
        r0 = half * ROWS
        ov = np.asarray(res.results[core]["o"]).reshape(NPART, C, WG, ROWS)
        for c in range(C):
            # ov[p, c, xl, y] = out[c, r0+y, 4p+xl]
            out[b, c, r0:r0 + ROWS, :] = ov[:, c].transpose(2, 0, 1).reshape(ROWS, W)
    return out
